# revision 32
# baseline (speedup 1.0000x reference)
"""GQA attention layer (B=2, S=2048, D=4096, 32 Q heads / 8 KV heads, RoPE,
causal) on 8 Trainium2 NeuronCores, tensor-parallel over heads.

Each core owns 4 Q heads + 1 KV head: it computes its Q/K/V projections,
RoPE, causal attention, and a partial output projection (rank-512 slice of
the wo contraction).  The host sums the 8 partial outputs.

v3 design (vs v2's 754us):
  * attention exp/pacc stage runs a fixed 2 chunks ahead of the AV stage,
    so the PE never waits on the 687ns ACT exp at unit boundaries (v2 lost
    ~1.2us per unit boundary re-filling the pt pipeline)
  * causal triangle applied as a 0/1 DVE multiply on the exp output instead
    of a PE matmul (-16k PE rows)
  * denominators: pacc accumulates all chunks (DVE), one ones-row matmul per
    unit emitted 2 chunks before unit end (-12k PE rows vs v2's 3 matmuls);
    the last head of each q-block uses a fast norm path (DVE reciprocal +
    fp32r PE broadcast + per-tcx copies/muls) so the output projection of
    the finished block can start draining immediately at the next unit
  * wo drains are paced over every chunk of every unit (v2 skipped the
    first 2 chunks of fresh units, bunching work at boundaries)
  * batch-1's V transposes are spread across projection groups 5-7 instead
    of bunched after the last group (v2 limped ~5us there)
  * startup: wk/wv/x chunk-0 DMAs issue first and each group's chunk stream
    is reordered [k,v,k,v,q,q,...] so the next group's k/v matmuls start
    while the q PSUM banks are still evicting
"""

import os
import sys
import types
from contextlib import ExitStack

import numpy as np
import ml_dtypes

import concourse.bass as bass
import concourse.tile as tile
from concourse import bacc
from concourse import mybir
from concourse import bass_utils
from concourse.bass_utils import run_bass_kernel_spmd

# ---------------------------------------------------------------------------
# Optional NTFF profiling support under axon. The trimmed image's `antenv`
# lacks `axon_hooks`, so run_bass_kernel_spmd(trace=True) would silently skip
# tracing; register the hook ourselves. Harmless when unavailable.
try:
    import antenv  # noqa: F401
    from trn_agent_boot.trn_boot import _ntff_profile_via_ctypes

    if "antenv.axon_hooks" not in sys.modules:
        _hooks_mod = types.ModuleType("antenv.axon_hooks")
        _hook = _ntff_profile_via_ctypes("/opt/axon/libaxon_pjrt.so")
        _hooks_mod.get_axon_ntff_profile_hook = lambda: _hook
        _hooks_mod.set_axon_ntff_profile_hook = lambda h: None
        sys.modules["antenv.axon_hooks"] = _hooks_mod
    bass_utils.upload_artifacts = lambda tmpdir: "local://skipped"
except Exception:
    pass

F32 = mybir.dt.float32
F32R = mybir.dt.float32r
BF16 = mybir.dt.bfloat16
EXP = mybir.ActivationFunctionType.Exp
NPBF16 = ml_dtypes.bfloat16

B, S, D = 2, 2048, 4096
NH, NKV, HD = 32, 8, 128
T = B * S                       # 4096 tokens total
N_CORES = 8
QH = NH // N_CORES              # 4 local q heads
FL = QH * HD                    # 512 local q features
SCALE = 1.0 / float(np.sqrt(HD))

NW = 512                        # token-group width in the QKV projection
QB = 512                        # q-block width in attention
DKD = D // 128                  # 32 contraction chunks for projections
NG = T // NW                    # 8 token groups


def _build_program():
    nc = bacc.Bacc("TRN2", target_bir_lowering=False, debug=False,
                   num_devices=N_CORES)

    xT = nc.dram_tensor("xT", [D, T], BF16, kind="ExternalInput").ap()
    wqT = nc.dram_tensor("wqT", [D, FL], BF16, kind="ExternalInput").ap()
    wkT = nc.dram_tensor("wkT", [D, HD], BF16, kind="ExternalInput").ap()
    wvT = nc.dram_tensor("wvT", [D, HD], BF16, kind="ExternalInput").ap()
    woT = nc.dram_tensor("woT", [FL, D], BF16, kind="ExternalInput").ap()
    # RoPE constants, pre-assembled for the rotate-half formulation on the
    # even/odd-split feature layout: ropc = [cos; cos], rops = [-sin; sin].
    ropc = nc.dram_tensor("ropc", [HD, S], BF16, kind="ExternalInput").ap()
    rops = nc.dram_tensor("rops", [HD, S], BF16, kind="ExternalInput").ap()
    idin = nc.dram_tensor("idin", [128, 128], BF16, kind="ExternalInput").ap()
    onesin = nc.dram_tensor("onesin", [128, 1], F32R, kind="ExternalInput").ap()
    onesrow = nc.dram_tensor("onesrow", [1, 128], F32R, kind="ExternalInput").ap()
    # Sub-diagonal 0/1 triangle: tri01[r, c] = 1 if c >= r else 0
    tri01in = nc.dram_tensor("tri01in", [128, 128], BF16, kind="ExternalInput").ap()
    y = nc.dram_tensor("y", [T, D], BF16, kind="ExternalOutput").ap()

    with tile.TileContext(nc) as tc, ExitStack() as ctx:
        # ------------------------------------------------------------------
        # Whole-program resident tiles: weights, constants, q/k/v activations
        # ------------------------------------------------------------------
        const = ctx.enter_context(tc.tile_pool(name="const", bufs=1))
        ident = const.tile([128, 128], BF16)
        ones_t = const.tile([128, 1], F32R)
        ones_row = const.tile([1, 128], F32R)
        tri01 = const.tile([128, 128], BF16)
        cos_s = const.tile([HD, S], BF16)
        sin_s = const.tile([HD, S], BF16)

        wpool = ctx.enter_context(tc.tile_pool(name="weights", bufs=1))
        # Resident weights, packed k-chunk-major: [128, DKD * width].
        # Chunk-interleaved DMAs so the first matmul group can start after
        # ~0.2 MiB instead of waiting for all 10 MiB of weights.
        wq_sb = wpool.tile([128, DKD * FL], BF16, tag="wq")
        wk_sb = wpool.tile([128, DKD * HD], BF16, tag="wk")
        wv_sb = wpool.tile([128, DKD * HD], BF16, tag="wv")
        wo_sb = wpool.tile([128, QH * D], BF16, tag="wo")

        def dma_w_chunk(j, k4=None):
            # k first: each group's chunk stream leads with the k matmul
            k4 = k4 if k4 is not None else slice(4 * j, 4 * j + 4)
            nc.sync.dma_start(
                wk_sb[:].rearrange("p (k f) -> p k f", k=DKD)[:, k4, :],
                wkT.rearrange("(k p) f -> p k f", p=128)[:, k4, :])
            nc.sync.dma_start(
                wv_sb[:].rearrange("p (k f) -> p k f", k=DKD)[:, k4, :],
                wvT.rearrange("(k p) f -> p k f", p=128)[:, k4, :])
            nc.sync.dma_start(
                wq_sb[:].rearrange("p (k f) -> p k f", k=DKD)[:, k4, :],
                wqT.rearrange("(k p) f -> p k f", p=128)[:, k4, :])

        vtpool = ctx.enter_context(tc.tile_pool(name="vtrans", bufs=2))
        ptpool = ctx.enter_context(tc.tile_pool(name="ptiles", bufs=6))
        # resident staging for group 7's deferred rope (see below); one xsw
        # buffer per head so the swap DMAs never WAR-block the sync queue
        g7pool = ctx.enter_context(tc.tile_pool(name="g7stage", bufs=1))
        g7xp = ctx.enter_context(tc.tile_pool(name="g7x", bufs=5))
        g7tmp = ctx.enter_context(tc.tile_pool(name="g7tmp", bufs=2))
        g7_staged = []
        resid = ctx.enter_context(tc.tile_pool(name="resid", bufs=1))
        q_res = [[resid.tile([128, S], BF16, tag=f"q{b}_{m}", name=f"q{b}_{m}")
                  for m in range(QH)] for b in range(B)]
        k_res = [resid.tile([128, S], BF16, tag=f"k{b}", name=f"k{b}")
                 for b in range(B)]
        v_res = [resid.tile([128, S], BF16, tag=f"v{b}", name=f"v{b}")
                 for b in range(B)]
        V_bs = [vtpool.tile([128, S], BF16, tag="V_b", name=f"V_{b}")
                for b in range(B)]

        # ------------------------------------------------------------------
        # Phase 1: QKV projections + RoPE -> resident SBUF (feature-major)
        # ------------------------------------------------------------------
        with tc.tile_pool(name="xin", bufs=3) as xpool, \
             tc.tile_pool(name="qkvstage", bufs=1) as stage, \
             tc.tile_pool(name="ropetmp", bufs=2) as rtmp, \
             tc.tile_pool(name="vtps", bufs=2, space="PSUM") as vtpsum, \
             tc.tile_pool(name="qkvps", bufs=1, space="PSUM") as qkvps:

            def rope_emit(src, dst, pos0, eng=None):
                """dst = RoPE(src) on the even/odd-split feature layout
                (partitions 0..63 even pair components, 64..127 odd):
                dst = src * [c;c] + swap_halves(src) * [-s;s]."""
                eng = eng or nc.vector
                c = cos_s[:, pos0:pos0 + NW]
                s = sin_s[:, pos0:pos0 + NW]
                xsw = rtmp.tile([128, NW], BF16, tag="xsw")
                nc.sync.dma_start(xsw[0:64, :], src[64:128, :])
                nc.sync.dma_start(xsw[64:128, :], src[0:64, :])
                t1 = rtmp.tile([128, NW], BF16, tag="t1")
                eng.tensor_mul(t1[:], xsw[:], s)
                t2 = rtmp.tile([128, NW], BF16, tag="t2")
                eng.tensor_mul(t2[:], src[:], c)
                eng.tensor_add(dst, t2[:], t1[:])

            def v_transpose(b, ch4):
                """Token-major copies V_bs[b][:, ch*128:+128] = v_res chunk.T
                for 4 position chunks, interleaved into the projection
                stream so the PE transposes hide behind other engines."""
                for ch in ch4:
                    vt_ps = vtpsum.tile([128, 128], BF16, tag="vtp",
                                        name="vtp")
                    nc.tensor.transpose(
                        vt_ps[:], v_res[b][:, ch * 128:(ch + 1) * 128],
                        ident[:])
                    if ch % 2 == 0:
                        nc.scalar.copy(
                            V_bs[b][:, ch * 128:(ch + 1) * 128], vt_ps[:])
                    else:
                        nc.vector.tensor_copy(
                            V_bs[b][:, ch * 128:(ch + 1) * 128], vt_ps[:])

            # Startup: smallest-possible first loads so the PE starts early:
            # wk/wv/wq chunk 0 (192 KiB), then x chunk 0 (128 KiB).
            xt_first = xpool.tile([128, 4 * NW], BF16)
            nc.sync.dma_start(
                xt_first[:].rearrange("p (k t) -> p k t", k=4)[:, 0:1, :],
                xT.rearrange("(k p) t -> p k t", p=128)[:, 0:1, 0:NW])
            dma_w_chunk(0, slice(0, 1))
            dma_w_chunk(0, slice(1, 4))

            for n in range(NG):
                b = n * NW // S
                pos0 = (n * NW) % S
                if n == 1:
                    nc.sync.dma_start(ident[:], idin)
                    nc.sync.dma_start(ones_t[:], onesin)
                    nc.sync.dma_start(ones_row[:], onesrow)
                    nc.sync.dma_start(tri01[:], tri01in)
                if 1 <= n <= 4:
                    # output-projection weights: prefetch in 1 MiB chunks
                    # during groups 1-4 so they never stall the x stream
                    fd = wo_sb[:].rearrange("p (f d) -> p f d", f=QH)
                    sd = woT.rearrange("(f p) d -> p f d", p=128)
                    nc.sync.dma_start(fd[:, n - 1:n, :], sd[:, n - 1:n, :])
                if n == 2:
                    # gpsimd ucode warm-up: the first partition_broadcast
                    # pays a ~7us program-load; absorb it here (gpsimd is
                    # idle all of phase 1) instead of at the first
                    # softmax normalization where the whole attention
                    # pipeline would convoy behind it
                    warm = rtmp.tile([128, NW], BF16, tag="xsw")
                    nc.gpsimd.partition_broadcast(warm[:], cos_s[0:1, 0:NW])
                qps = [qkvps.tile([128, NW], F32, tag=f"qps{m}", name=f"qps{m}")
                       for m in range(QH)]
                kps = qkvps.tile([128, NW], F32, tag="kps")
                vps = qkvps.tile([128, NW], F32, tag="vps")

                def mm_k(k, xt):
                    nc.tensor.matmul(
                        kps[:], wk_sb[:, k * HD:(k + 1) * HD], xt,
                        start=(k == 0), stop=(k == DKD - 1))

                def mm_v(k, xt):
                    nc.tensor.matmul(
                        vps[:], wv_sb[:, k * HD:(k + 1) * HD], xt,
                        start=(k == 0), stop=(k == DKD - 1))

                def mm_q(k, xt):
                    for m in range(QH):
                        nc.tensor.matmul(
                            qps[m][:],
                            wq_sb[:, k * FL + m * 128:k * FL + (m + 1) * 128],
                            xt, start=(k == 0), stop=(k == DKD - 1))

                for j in range(DKD // 4):
                    if n == 0 and j == 0:
                        xt4 = xt_first
                        nc.sync.dma_start(
                            xt4[:].rearrange("p (k t) -> p k t", k=4)[:, 1:4, :],
                            xT.rearrange("(k p) t -> p k t", p=128)[
                                :, 1:4, 0:NW])
                    else:
                        xt4 = xpool.tile([128, 4 * NW], BF16)
                        nc.sync.dma_start(
                            xt4[:].rearrange("p (k t) -> p k t", k=4),
                            xT.rearrange("(k p) t -> p k t", p=128)[
                                :, 4 * j:4 * j + 4, n * NW:(n + 1) * NW])
                    if n == 0 and j < 7:
                        dma_w_chunk(j + 1)
                    if n == 0 and j == 7:
                        # needed by this group's RoPE, after the last x tile
                        nc.sync.dma_start(cos_s[:], ropc)
                        nc.sync.dma_start(sin_s[:], rops)
                    # V transposes ride the groups' streams, at most 2 per
                    # 4-chunk step so their PSUM evictions never gate the PE:
                    # batch 0's chunks during groups 4-7, batch 1's first 12
                    # during groups 5-7 (the last 4 move into phase 2).
                    if n == 4 and 2 <= j <= 5:
                        v_transpose(0, [j - 2])
                    if n >= 5 and j in (1, 3, 5, 7):
                        v_transpose(0, [(n - 4) * 4 + (j - 1) // 2])
                    if n >= 5 and j in (2, 4, 6):
                        chs = [(n - 5) * 4 + j // 2 - 1]
                        if j == 6:
                            chs.append((n - 5) * 4 + 3)
                        v_transpose(1, chs)
                    xts = [xt4[:, kk * NW:(kk + 1) * NW] for kk in range(4)]
                    if j == 0:
                        # group boundary: k/v of chunks 0-1 first so the PE
                        # has work while the q PSUM banks finish evicting
                        mm_k(4 * j + 0, xts[0])
                        mm_v(4 * j + 0, xts[0])
                        mm_k(4 * j + 1, xts[1])
                        mm_v(4 * j + 1, xts[1])
                        mm_q(4 * j + 0, xts[0])
                        mm_q(4 * j + 1, xts[1])
                        rest = (2, 3)
                    else:
                        rest = (0, 1, 2, 3)
                    for kk in rest:
                        k = 4 * j + kk
                        mm_k(k, xts[kk])
                        mm_v(k, xts[kk])
                        mm_q(k, xts[kk])
                # Evict all 6 PSUM accumulators first (frees banks for the
                # next group ASAP; k first since the next group's stream
                # leads with k), alternating ACT/DVE; then RoPE math.
                spool = g7pool if n == NG - 1 else stage
                kc = spool.tile([128, NW], BF16, tag="kc")
                nc.scalar.copy(kc[:], kps[:])
                # v needs no RoPE: cast straight into the resident tile.
                nc.vector.tensor_copy(v_res[b][:, pos0:pos0 + NW], vps[:])
                qc = []
                for m in range(QH):
                    t = spool.tile([128, NW], BF16, tag=f"qc{m}", name=f"qc{m}")
                    if m % 2 == 0:
                        nc.scalar.copy(t[:], qps[m][:])
                    else:
                        nc.vector.tensor_copy(t[:], qps[m][:])
                    qc.append(t)
                if n < NG - 1:
                    for m in range(QH):
                        rope_emit(qc[m], q_res[b][m][:, pos0:pos0 + NW], pos0)
                    rope_emit(kc, k_res[b][:, pos0:pos0 + NW], pos0)
                else:
                    # group 7's rope is deferred into phase 2 (the staging
                    # tiles are resident): its results aren't needed until
                    # the batch-1 attention units ~200us later, and keeping
                    # its 15 DVE ops off the queue here lets the first
                    # attention units' DVE work start immediately
                    g7_staged.extend(
                        [(qc[m], q_res[1][m]) for m in range(QH)]
                        + [(kc, k_res[1])])

        # ------------------------------------------------------------------
        # Phase 2: attention + output projection
        # ------------------------------------------------------------------
        with tc.tile_pool(name="pacc", bufs=2) as papool, \
             tc.tile_pool(name="attn", bufs=2) as atpool, \
             tc.tile_pool(name="smax", bufs=2) as smpool, \
             tc.tile_pool(name="ystage", bufs=2) as ypool, \
             tc.tile_pool(name="sps", bufs=3, space="PSUM") as spsum, \
             tc.tile_pool(name="avps", bufs=2, space="PSUM") as avpsum, \
             tc.tile_pool(name="normps", bufs=1, space="PSUM") as normps, \
             tc.tile_pool(name="yps", bufs=2, space="PSUM") as ypsum:

            def wo_gen(att_prev, b_prev, q0_prev, fine_dma=False):
                """Output projection for a finished q block, as a generator
                that yields once per PE matmul so the caller can interleave
                them into the attention stream."""
                for tcx in range(QB // 128):
                    tg0 = b_prev * S + q0_prev + tcx * 128
                    for half in range(2):
                        ysb = ypool.tile([128, D // 2], BF16, tag="ysb",
                                         name="ysb")
                        for dgh in range(4):
                            dg = half * 4 + dgh
                            yp = ypsum.tile([128, NW], F32, tag="yp", name="yp")
                            for f in range(QH):
                                nc.tensor.matmul(
                                    yp[:],
                                    att_prev[f][:, tcx * 128:(tcx + 1) * 128],
                                    wo_sb[:, f * D + dg * NW:f * D + (dg + 1) * NW],
                                    start=(f == 0), stop=(f == QH - 1))
                                yield
                            # half on ACT, half on DVE: shorter queue items
                            # mean the exp stream is never blocked >370ns,
                            # and the yp bank frees twice as fast
                            nc.scalar.copy(
                                ysb[:, dgh * NW:dgh * NW + NW // 2],
                                yp[:, 0:NW // 2])
                            nc.vector.tensor_copy(
                                ysb[:, dgh * NW + NW // 2:(dgh + 1) * NW],
                                yp[:, NW // 2:NW])
                            if fine_dma:
                                # tail of the kernel: ship each 512-col piece
                                # as soon as it's staged
                                nc.sync.dma_start(
                                    y[tg0:tg0 + 128,
                                      dg * NW:(dg + 1) * NW],
                                    ysb[:, dgh * NW:(dgh + 1) * NW])
                        if not fine_dma:
                            nc.sync.dma_start(
                                y[tg0:tg0 + 128,
                                  half * (D // 2):(half + 1) * (D // 2)],
                                ysb[:])

            def drain(gen, k):
                if gen is None:
                    return
                for _ in range(k):
                    try:
                        next(gen)
                    except StopIteration:
                        return

            units = [(b, qb, h) for b in range(B)
                     for qb in range(S // QB) for h in range(QH)]

            def u_nkt(u):
                return (u[1] + 1) * (QB // 128)

            def chunk_geom(u, c):
                vv = c - (u_nkt(u) - 4)
                if vv >= 0:
                    return vv * 128, (4 - vv) * 128  # qoff, width
                return 0, QB

            flat = [(i, c) for i, u in enumerate(units)
                    for c in range(u_nkt(u))]
            score_tiles = {}
            pt_tiles = {}
            pacc_cur = {}

            def emit_score(i, c):
                ub, uqb, uh = units[i]
                qoff, w = chunk_geom(units[i], c)
                t = spsum.tile([128, QB], F32, tag="stp", name="stp")
                nc.tensor.matmul(
                    t[:, 0:w], k_res[ub][:, c * 128:(c + 1) * 128],
                    q_res[ub][uh][:, uqb * QB + qoff:uqb * QB + qoff + w],
                    start=True, stop=True)
                score_tiles[(i, c)] = t

            def emit_score_pos(p):
                if p < len(flat):
                    i, c = flat[p]
                    if (i, c) not in score_tiles:
                        emit_score(i, c)

            def exp_stage(p):
                """exp + causal mask + denominator accumulation for flat[p].
                Runs 2 chunks ahead of the AV stage so the 687ns ACT exp is
                never on the PE's critical path."""
                if p >= len(flat):
                    return
                i2, c2 = flat[p]
                u2 = units[i2]
                nkt2 = u_nkt(u2)
                qoff2, w2 = chunk_geom(u2, c2)
                if (i2, c2) not in score_tiles:
                    emit_score(i2, c2)
                stp = score_tiles.pop((i2, c2))
                pt = ptpool.tile([128, QB], BF16, tag="pt", name="pt")
                nc.scalar.activation(pt[:, 0:w2], stp[:, 0:w2], EXP,
                                     scale=SCALE)
                if c2 >= nkt2 - 4:
                    # causal triangle: zero the below-diagonal block of the
                    # first 128 q columns (0/1 multiply on the DVE keeps the
                    # PE out of it entirely)
                    nc.vector.tensor_mul(pt[:, 0:128], pt[:, 0:128], tri01[:])
                if c2 == 0:
                    pacc_cur[i2] = papool.tile([128, QB], F32R, tag="pacc",
                                               name="pacc")
                    nc.vector.tensor_copy(pacc_cur[i2][:], pt[:])
                else:
                    nc.vector.tensor_add(
                        pacc_cur[i2][:, qoff2:qoff2 + w2],
                        pacc_cur[i2][:, qoff2:qoff2 + w2], pt[:, 0:w2])
                pt_tiles[(i2, c2)] = pt

            # gpsimd-broadcast norm path, used for heads 0..QH-2 where the
            # whole 4-engine chain hides inside the next unit's stream.
            def make_norm(avp, smp, att_t):
                def norm():
                    s1 = smpool.tile([1, QB], F32, tag="s1", name="s1")
                    nc.vector.tensor_copy(s1[:], smp[0:1, :])
                    s_bc = smpool.tile([128, QB], F32, tag="s_bc")
                    nc.gpsimd.partition_broadcast(s_bc[:], s1[:])
                    r_bc = smpool.tile([128, QB], F32, tag="r_bc")
                    nc.vector.reciprocal_approx_fast(r_bc[:], s_bc[:])
                    nc.vector.tensor_mul(att_t[:], avp[:], r_bc[:])
                return norm

            # Group 7's deferred rope: all 10 half-swap DMAs issue upfront
            # (sync queue is empty here, and each head has its own xsw
            # buffer so there are no WAR waits); the 15 DVE ops then drip
            # in one per two attention iterations so they never back up the
            # DVE queue. Results are needed ~200us later (batch 1).
            g7_ops = []
            g7_pos0 = ((NG - 1) * NW) % S
            for g7_src, g7_dst in g7_staged:
                g7_xsw = g7xp.tile([128, NW], BF16, tag="xsw")
                nc.sync.dma_start(g7_xsw[0:64, :], g7_src[64:128, :])
                nc.sync.dma_start(g7_xsw[64:128, :], g7_src[0:64, :])
                g7_ref = {}

                def op_a(xsw=g7_xsw, ref=g7_ref):
                    t1 = g7tmp.tile([128, NW], BF16, tag="t1")
                    nc.vector.tensor_mul(t1[:], xsw[:],
                                         sin_s[:, g7_pos0:g7_pos0 + NW])
                    ref["t1"] = t1

                def op_b(src=g7_src, ref=g7_ref):
                    t2 = g7tmp.tile([128, NW], BF16, tag="t2")
                    nc.vector.tensor_mul(t2[:], src[:],
                                         cos_s[:, g7_pos0:g7_pos0 + NW])
                    ref["t2"] = t2

                def op_c(dst=g7_dst, ref=g7_ref):
                    nc.vector.tensor_add(dst[:, g7_pos0:g7_pos0 + NW],
                                         ref["t2"][:], ref["t1"][:])

                g7_ops.extend([op_a, op_b, op_c])

            pending = None
            deferred = None
            att = None
            emit_score_pos(0)
            emit_score_pos(1)
            emit_score_pos(2)
            exp_stage(0)
            exp_stage(1)

            pos = 0
            avp = None
            smp = None
            r1 = None
            rbc_ps = None
            for i, u in enumerate(units):
                b, qb, h = u
                nkt = u_nkt(u)
                V_b = V_bs[b]
                last_head = (h == QH - 1)
                if h == 0:
                    att = [atpool.tile([128, QB], BF16, tag=f"att{hh}",
                                       name=f"att{hh}") for hh in range(QH)]
                wo_per_chunk = -(-(QB // 128 * 8) // nkt)   # ceil

                for c in range(nkt):
                    exp_stage(pos + 2)
                    if c == 0:
                        avp = avpsum.tile([128, QB], F32, tag="avp",
                                          name="avp")
                    pt_t = pt_tiles.pop((i, c))
                    qoff, w = chunk_geom(u, c)
                    nc.tensor.matmul(
                        avp[:, qoff:qoff + w],
                        V_b[:, c * 128:(c + 1) * 128], pt_t[:, 0:w],
                        start=(c == 0), stop=(c == nkt - 1))
                    emit_score_pos(pos + 3)
                    if i == 0 and c < 4:
                        # batch 1's last 4 V transposes, deferred into the
                        # first attention unit: this stretch is ACT-bound
                        # (no wo drains yet) so the PE has spare cycles,
                        # and it shortens the phase-1 tail
                        if c == 0:
                            vt_sp = avpsum.tile([128, QB], BF16, tag="avp",
                                                name="avp")
                        sl = slice(c * 128, (c + 1) * 128)
                        nc.tensor.transpose(
                            vt_sp[:, sl],
                            v_res[1][:, (12 + c) * 128:(13 + c) * 128],
                            ident[:])
                        if c % 2 == 0:
                            nc.scalar.copy(
                                V_bs[1][:, (12 + c) * 128:(13 + c) * 128],
                                vt_sp[:, sl])
                        else:
                            nc.vector.tensor_copy(
                                V_bs[1][:, (12 + c) * 128:(13 + c) * 128],
                                vt_sp[:, sl])
                    if c == 1 and deferred is not None:
                        deferred()
                        deferred = None
                    if c == (nkt - 3 if last_head else nkt - 1):
                        # pacc is complete already (exp stage ran 2 ahead):
                        # denominator row-sums via one 512-row matmul. For
                        # non-last heads this sits at unit end so it never
                        # stalls the PE queue on a backlogged DVE (the
                        # deferred norm only needs it one unit later).
                        smp = normps.tile([128, QB], F32, tag="np", name="np")
                        nc.tensor.matmul(
                            smp[0:1, :], ones_t[:], pacc_cur[i][:],
                            start=True, stop=True)
                    if c == nkt - 2 and last_head:
                        # fast norm path pieces 1+2: stage the row sums to
                        # SBUF (fp32r), then an fp32r PE outer-product
                        # broadcast of the *denominator*
                        r1 = smpool.tile([1, QB], F32R, tag="s1", name="s1")
                        nc.vector.tensor_copy(r1[:], smp[0:1, :])
                        rbc_ps = normps.tile([128, QB], F32, tag="np",
                                             name="np")
                        nc.tensor.matmul(
                            rbc_ps[:], ones_row[:], r1[:],
                            start=True, stop=True)
                    drain(pending, wo_per_chunk)
                    # start only after the swap DMAs are certainly resident
                    # (batch-1 units begin at pos 160; last op lands ~119)
                    if g7_ops and pos >= 32 and pos % 6 == 5:
                        g7_ops.pop(0)()
                    pos += 1
                if deferred is not None:
                    deferred()
                    deferred = None
                if not last_head:
                    deferred = make_norm(avp, smp, att[h])
                else:
                    # fast norm path piece 3: per-tcx reciprocal+mul so the
                    # first 128 columns of att[3] are ready ~1.8us after the
                    # last AV instead of ~2.9us via the gpsimd chain
                    rbc_sb = smpool.tile([128, QB], F32, tag="r_bc")
                    for t4 in range(4):
                        sl = slice(t4 * 128, (t4 + 1) * 128)
                        nc.vector.reciprocal_approx_fast(rbc_sb[:, sl],
                                                         rbc_ps[:, sl])
                        nc.vector.tensor_mul(att[h][:, sl], avp[:, sl],
                                             rbc_sb[:, sl])
                    drain(pending, 10 ** 9)
                    pending = wo_gen(att, b, qb * QB,
                                     fine_dma=(i == len(units) - 1))
            drain(pending, 10 ** 9)
    nc.compile()
    return nc


_program = None


def _get_program():
    global _program
    if _program is None:
        _program = _build_program()
    return _program


def kernel(**inputs) -> np.ndarray:
    x = np.asarray(inputs["x"], dtype=np.float32)
    wq = np.asarray(inputs["wq"], dtype=np.float32)
    wk = np.asarray(inputs["wk"], dtype=np.float32)
    wv = np.asarray(inputs["wv"], dtype=np.float32)
    wo = np.asarray(inputs["wo"], dtype=np.float32)
    cos = np.asarray(inputs["freqs_cos"], dtype=np.float32)
    sin = np.asarray(inputs["freqs_sin"], dtype=np.float32)
    start_pos = int(np.asarray(inputs.get("start_pos", 0)))
    assert start_pos == 0, "kernel specialized for start_pos == 0"

    # Even/odd RoPE pair split within each head's 128 features.
    perm = np.concatenate([np.arange(0, HD, 2), np.arange(1, HD, 2)])

    xT = np.ascontiguousarray(x.reshape(T, D).T.astype(NPBF16))
    cosT = cos.T                                   # [64, S]
    sinT = sin.T
    ropc = np.ascontiguousarray(
        np.concatenate([cosT, cosT], axis=0).astype(NPBF16))
    rops = np.ascontiguousarray(
        np.concatenate([-sinT, sinT], axis=0).astype(NPBF16))
    rr, cc = np.meshgrid(np.arange(128), np.arange(128), indexing="ij")
    tri01in = (cc >= rr).astype(np.float32).astype(NPBF16)

    in_maps = []
    for c in range(N_CORES):
        wq_c = wq[c * FL:(c + 1) * FL].reshape(QH, HD, D)[:, perm, :].reshape(FL, D)
        wk_c = wk[c * HD:(c + 1) * HD][perm, :]
        wv_c = wv[c * HD:(c + 1) * HD]
        wo_c = wo[:, c * FL:(c + 1) * FL]
        in_maps.append({
            "xT": xT,
            "idin": np.eye(128, dtype=np.float32).astype(NPBF16),
            "wqT": np.ascontiguousarray(wq_c.T.astype(NPBF16)),
            "wkT": np.ascontiguousarray(wk_c.T.astype(NPBF16)),
            "wvT": np.ascontiguousarray(wv_c.T.astype(NPBF16)),
            "woT": np.ascontiguousarray(wo_c.T.astype(NPBF16)),
            "ropc": ropc,
            "rops": rops,
            "onesin": np.ones((128, 1), dtype=np.float32),
            "onesrow": np.ones((1, 128), dtype=np.float32),
            "tri01in": tri01in,
        })

    nc = _get_program()
    trace = bool(int(os.environ.get("GQA_TRACE", "0")))
    kwargs = {}
    if trace:
        tmpdir = os.environ.get("GQA_TRACE_DIR") or None
        kwargs = dict(trace=True, tmpdir=tmpdir, trace_cores=[0])
    res = run_bass_kernel_spmd(nc, in_maps, list(range(N_CORES)), **kwargs)
    kernel.last_results = res

    acc = np.zeros((T, D), dtype=np.float32)
    for c in range(N_CORES):
        acc += np.asarray(res.results[c]["y"]).astype(np.float32)
    return acc.reshape(B, S, D)


# revision 39
# speedup vs baseline: 1.0238x; 1.0238x over previous
"""GQA attention layer (B=2, S=2048, D=4096, 32 Q heads / 8 KV heads, RoPE,
causal) on 8 Trainium2 NeuronCores, tensor-parallel over heads.

Each core owns 4 Q heads + 1 KV head: it computes its Q/K/V projections,
RoPE, causal attention, and a partial output projection (rank-512 slice of
the wo contraction).  The host sums the 8 partial outputs.

v3 design (vs v2's 754us):
  * attention exp/pacc stage runs a fixed 2 chunks ahead of the AV stage,
    so the PE never waits on the 687ns ACT exp at unit boundaries (v2 lost
    ~1.2us per unit boundary re-filling the pt pipeline)
  * causal triangle applied as a 0/1 DVE multiply on the exp output instead
    of a PE matmul (-16k PE rows)
  * denominators: pacc accumulates all chunks (DVE), one ones-row matmul per
    unit emitted 2 chunks before unit end (-12k PE rows vs v2's 3 matmuls);
    the last head of each q-block uses a fast norm path (DVE reciprocal +
    fp32r PE broadcast + per-tcx copies/muls) so the output projection of
    the finished block can start draining immediately at the next unit
  * wo drains are paced over every chunk of every unit (v2 skipped the
    first 2 chunks of fresh units, bunching work at boundaries)
  * batch-1's V transposes are spread across projection groups 5-7 instead
    of bunched after the last group (v2 limped ~5us there)
  * startup: wk/wv/x chunk-0 DMAs issue first and each group's chunk stream
    is reordered [k,v,k,v,q,q,...] so the next group's k/v matmuls start
    while the q PSUM banks are still evicting
"""

import os
import sys
import types
from contextlib import ExitStack

import numpy as np
import ml_dtypes

import concourse.bass as bass
import concourse.tile as tile
from concourse import bacc
from concourse import mybir
from concourse import bass_utils
from concourse.bass_utils import run_bass_kernel_spmd

# ---------------------------------------------------------------------------
# Optional NTFF profiling support under axon. The trimmed image's `antenv`
# lacks `axon_hooks`, so run_bass_kernel_spmd(trace=True) would silently skip
# tracing; register the hook ourselves. Harmless when unavailable.
try:
    import antenv  # noqa: F401
    from trn_agent_boot.trn_boot import _ntff_profile_via_ctypes

    if "antenv.axon_hooks" not in sys.modules:
        _hooks_mod = types.ModuleType("antenv.axon_hooks")
        _hook = _ntff_profile_via_ctypes("/opt/axon/libaxon_pjrt.so")
        _hooks_mod.get_axon_ntff_profile_hook = lambda: _hook
        _hooks_mod.set_axon_ntff_profile_hook = lambda h: None
        sys.modules["antenv.axon_hooks"] = _hooks_mod
    bass_utils.upload_artifacts = lambda tmpdir: "local://skipped"
except Exception:
    pass

F32 = mybir.dt.float32
F32R = mybir.dt.float32r
BF16 = mybir.dt.bfloat16
EXP = mybir.ActivationFunctionType.Exp
NPBF16 = ml_dtypes.bfloat16

B, S, D = 2, 2048, 4096
NH, NKV, HD = 32, 8, 128
T = B * S                       # 4096 tokens total
N_CORES = 8
QH = NH // N_CORES              # 4 local q heads
FL = QH * HD                    # 512 local q features
SCALE = 1.0 / float(np.sqrt(HD))

NW = 512                        # token-group width in the QKV projection
QB = 512                        # q-block width in attention
DKD = D // 128                  # 32 contraction chunks for projections
NG = T // NW                    # 8 token groups


def _build_program():
    nc = bacc.Bacc("TRN2", target_bir_lowering=False, debug=False,
                   num_devices=N_CORES)

    xT = nc.dram_tensor("xT", [D, T], BF16, kind="ExternalInput").ap()
    wqT = nc.dram_tensor("wqT", [D, FL], BF16, kind="ExternalInput").ap()
    wkT = nc.dram_tensor("wkT", [D, HD], BF16, kind="ExternalInput").ap()
    wvT = nc.dram_tensor("wvT", [D, HD], BF16, kind="ExternalInput").ap()
    woT = nc.dram_tensor("woT", [FL, D], BF16, kind="ExternalInput").ap()
    # RoPE constants, pre-assembled for the rotate-half formulation on the
    # even/odd-split feature layout: ropc = [cos; cos], rops = [-sin; sin].
    ropc = nc.dram_tensor("ropc", [HD, S], BF16, kind="ExternalInput").ap()
    rops = nc.dram_tensor("rops", [HD, S], BF16, kind="ExternalInput").ap()
    idin = nc.dram_tensor("idin", [128, 128], BF16, kind="ExternalInput").ap()
    onesin = nc.dram_tensor("onesin", [128, 1], BF16, kind="ExternalInput").ap()
    onesrow = nc.dram_tensor("onesrow", [1, 128], F32R, kind="ExternalInput").ap()
    # Sub-diagonal 0/1 triangle: tri01[r, c] = 1 if c >= r else 0
    tri01in = nc.dram_tensor("tri01in", [128, 128], BF16, kind="ExternalInput").ap()
    y = nc.dram_tensor("y", [T, D], BF16, kind="ExternalOutput").ap()

    with tile.TileContext(nc) as tc, ExitStack() as ctx:
        # ------------------------------------------------------------------
        # Whole-program resident tiles: weights, constants, q/k/v activations
        # ------------------------------------------------------------------
        const = ctx.enter_context(tc.tile_pool(name="const", bufs=1))
        ident = const.tile([128, 128], BF16)
        gwarm = const.tile([128, 1], BF16)
        ones_t = const.tile([128, 1], BF16)
        ones_row = const.tile([1, 128], F32R)
        tri01 = const.tile([128, 128], BF16)
        cos_s = const.tile([HD, S], BF16)
        sin_s = const.tile([HD, S], BF16)

        wpool = ctx.enter_context(tc.tile_pool(name="weights", bufs=1))
        # Resident weights, packed k-chunk-major: [128, DKD * width].
        # Chunk-interleaved DMAs so the first matmul group can start after
        # ~0.2 MiB instead of waiting for all 10 MiB of weights.
        wq_sb = wpool.tile([128, DKD * FL], BF16, tag="wq")
        wk_sb = wpool.tile([128, DKD * HD], BF16, tag="wk")
        wv_sb = wpool.tile([128, DKD * HD], BF16, tag="wv")
        wo_sb = wpool.tile([128, QH * D], BF16, tag="wo")

        def dma_w_chunk(j, k4=None):
            # k first: each group's chunk stream leads with the k matmul
            k4 = k4 if k4 is not None else slice(4 * j, 4 * j + 4)
            nc.sync.dma_start(
                wk_sb[:].rearrange("p (k f) -> p k f", k=DKD)[:, k4, :],
                wkT.rearrange("(k p) f -> p k f", p=128)[:, k4, :])
            nc.sync.dma_start(
                wv_sb[:].rearrange("p (k f) -> p k f", k=DKD)[:, k4, :],
                wvT.rearrange("(k p) f -> p k f", p=128)[:, k4, :])
            nc.sync.dma_start(
                wq_sb[:].rearrange("p (k f) -> p k f", k=DKD)[:, k4, :],
                wqT.rearrange("(k p) f -> p k f", p=128)[:, k4, :])

        vtpool = ctx.enter_context(tc.tile_pool(name="vtrans", bufs=2))
        ptpool = ctx.enter_context(tc.tile_pool(name="ptiles", bufs=6))
        # resident staging for group 7's deferred rope (see below); one xsw
        # buffer per head so the swap DMAs never WAR-block the sync queue
        g7pool = ctx.enter_context(tc.tile_pool(name="g7stage", bufs=1))
        g7xp = ctx.enter_context(tc.tile_pool(name="g7x", bufs=5))
        g7tmp = ctx.enter_context(tc.tile_pool(name="g7tmp", bufs=2))
        g7_staged = []
        resid = ctx.enter_context(tc.tile_pool(name="resid", bufs=1))
        q_res = [[resid.tile([128, S], BF16, tag=f"q{b}_{m}", name=f"q{b}_{m}")
                  for m in range(QH)] for b in range(B)]
        k_res = [resid.tile([128, S], BF16, tag=f"k{b}", name=f"k{b}")
                 for b in range(B)]
        v_res = [resid.tile([128, S], BF16, tag=f"v{b}", name=f"v{b}")
                 for b in range(B)]
        V_bs = [vtpool.tile([128, S], BF16, tag="V_b", name=f"V_{b}")
                for b in range(B)]

        # ------------------------------------------------------------------
        # Phase 1: QKV projections + RoPE -> resident SBUF (feature-major)
        # ------------------------------------------------------------------
        with tc.tile_pool(name="xin", bufs=3) as xpool, \
             tc.tile_pool(name="qkvstage", bufs=1) as stage, \
             tc.tile_pool(name="ropetmp", bufs=2) as rtmp, \
             tc.tile_pool(name="vtps", bufs=2, space="PSUM") as vtpsum, \
             tc.tile_pool(name="qkvps", bufs=1, space="PSUM") as qkvps:

            def rope_emit(src, dst, pos0, eng=None):
                """dst = RoPE(src) on the even/odd-split feature layout
                (partitions 0..63 even pair components, 64..127 odd):
                dst = src * [c;c] + swap_halves(src) * [-s;s]."""
                eng = eng or nc.vector
                c = cos_s[:, pos0:pos0 + NW]
                s = sin_s[:, pos0:pos0 + NW]
                xsw = rtmp.tile([128, NW], BF16, tag="xsw")
                nc.sync.dma_start(xsw[0:64, :], src[64:128, :])
                nc.sync.dma_start(xsw[64:128, :], src[0:64, :])
                t1 = rtmp.tile([128, NW], BF16, tag="t1")
                eng.tensor_mul(t1[:], xsw[:], s)
                t2 = rtmp.tile([128, NW], BF16, tag="t2")
                eng.tensor_mul(t2[:], src[:], c)
                eng.tensor_add(dst, t2[:], t1[:])

            def v_transpose(b, ch4):
                """Token-major copies V_bs[b][:, ch*128:+128] = v_res chunk.T
                for 4 position chunks, interleaved into the projection
                stream so the PE transposes hide behind other engines."""
                for ch in ch4:
                    vt_ps = vtpsum.tile([128, 128], BF16, tag="vtp",
                                        name="vtp")
                    nc.tensor.transpose(
                        vt_ps[:], v_res[b][:, ch * 128:(ch + 1) * 128],
                        ident[:])
                    if ch % 2 == 0:
                        nc.scalar.copy(
                            V_bs[b][:, ch * 128:(ch + 1) * 128], vt_ps[:])
                    else:
                        nc.vector.tensor_copy(
                            V_bs[b][:, ch * 128:(ch + 1) * 128], vt_ps[:])

            # Startup: smallest-possible first loads so the PE starts early:
            # wk/wv/wq chunk 0 (192 KiB), then x chunk 0 (128 KiB).
            xt_first = xpool.tile([128, 4 * NW], BF16)
            nc.sync.dma_start(
                xt_first[:].rearrange("p (k t) -> p k t", k=4)[:, 0:1, :],
                xT.rearrange("(k p) t -> p k t", p=128)[:, 0:1, 0:NW])
            dma_w_chunk(0, slice(0, 1))
            dma_w_chunk(0, slice(1, 4))

            for n in range(NG):
                b = n * NW // S
                pos0 = (n * NW) % S
                if n == 1:
                    nc.sync.dma_start(ident[:], idin)
                    nc.sync.dma_start(ones_t[:], onesin)
                    nc.sync.dma_start(ones_row[:], onesrow)
                    nc.sync.dma_start(tri01[:], tri01in)
                if 1 <= n <= 4:
                    # output-projection weights: prefetch in 1 MiB chunks
                    # during groups 1-4 so they never stall the x stream
                    fd = wo_sb[:].rearrange("p (f d) -> p f d", f=QH)
                    sd = woT.rearrange("(f p) d -> p f d", p=128)
                    nc.sync.dma_start(fd[:, n - 1:n, :], sd[:, n - 1:n, :])
                if n == 2:
                    # gpsimd ucode warm-up: the first partition_broadcast
                    # pays a ~7us program-load; absorb it here (gpsimd is
                    # idle all of phase 1) instead of at the first softmax
                    # normalization where the whole attention pipeline
                    # would convoy behind it. Dedicated output tile so no
                    # pool rotation ever waits on the slow load.
                    nc.gpsimd.partition_broadcast(gwarm[:], cos_s[0:1, 0:1])
                qps = [qkvps.tile([128, NW], F32, tag=f"qps{m}", name=f"qps{m}")
                       for m in range(QH)]
                kps = qkvps.tile([128, NW], F32, tag="kps")
                vps = qkvps.tile([128, NW], F32, tag="vps")

                def mm_k(k, xt):
                    nc.tensor.matmul(
                        kps[:], wk_sb[:, k * HD:(k + 1) * HD], xt,
                        start=(k == 0), stop=(k == DKD - 1))

                def mm_v(k, xt):
                    nc.tensor.matmul(
                        vps[:], wv_sb[:, k * HD:(k + 1) * HD], xt,
                        start=(k == 0), stop=(k == DKD - 1))

                def mm_q(k, xt):
                    for m in range(QH):
                        nc.tensor.matmul(
                            qps[m][:],
                            wq_sb[:, k * FL + m * 128:k * FL + (m + 1) * 128],
                            xt, start=(k == 0), stop=(k == DKD - 1))

                for j in range(DKD // 4):
                    if n == 0 and j == 0:
                        xt4 = xt_first
                        nc.sync.dma_start(
                            xt4[:].rearrange("p (k t) -> p k t", k=4)[:, 1:4, :],
                            xT.rearrange("(k p) t -> p k t", p=128)[
                                :, 1:4, 0:NW])
                    else:
                        xt4 = xpool.tile([128, 4 * NW], BF16)
                        nc.sync.dma_start(
                            xt4[:].rearrange("p (k t) -> p k t", k=4),
                            xT.rearrange("(k p) t -> p k t", p=128)[
                                :, 4 * j:4 * j + 4, n * NW:(n + 1) * NW])
                    if n == 0 and j < 7:
                        dma_w_chunk(j + 1)
                    if n == 0 and j == 7:
                        # needed by this group's RoPE, after the last x tile
                        nc.sync.dma_start(cos_s[:], ropc)
                        nc.sync.dma_start(sin_s[:], rops)
                    # V transposes ride the groups' streams, at most 2 per
                    # 4-chunk step so their PSUM evictions never gate the PE:
                    # batch 0's chunks during groups 4-7, batch 1's first 12
                    # during groups 5-7 (the last 4 move into phase 2).
                    if n == 4 and 2 <= j <= 5:
                        v_transpose(0, [j - 2])
                    if n >= 5 and j in (1, 3, 5, 7):
                        v_transpose(0, [(n - 4) * 4 + (j - 1) // 2])
                    if n >= 5 and j in (2, 4, 6):
                        chs = [(n - 5) * 4 + j // 2 - 1]
                        if j == 6:
                            chs.append((n - 5) * 4 + 3)
                        v_transpose(1, chs)
                    xts = [xt4[:, kk * NW:(kk + 1) * NW] for kk in range(4)]
                    if j == 0:
                        # group boundary: k/v of chunks 0-1 first so the PE
                        # has work while the q PSUM banks finish evicting
                        mm_k(4 * j + 0, xts[0])
                        mm_v(4 * j + 0, xts[0])
                        mm_k(4 * j + 1, xts[1])
                        mm_v(4 * j + 1, xts[1])
                        mm_q(4 * j + 0, xts[0])
                        mm_q(4 * j + 1, xts[1])
                        rest = (2, 3)
                    else:
                        rest = (0, 1, 2, 3)
                    for kk in rest:
                        k = 4 * j + kk
                        mm_k(k, xts[kk])
                        mm_v(k, xts[kk])
                        mm_q(k, xts[kk])
                # Evict all 6 PSUM accumulators first (frees banks for the
                # next group ASAP; k first since the next group's stream
                # leads with k), alternating ACT/DVE; then RoPE math.
                spool = g7pool if n == NG - 1 else stage
                kc = spool.tile([128, NW], BF16, tag="kc")
                nc.scalar.copy(kc[:], kps[:])
                # v needs no RoPE: cast straight into the resident tile.
                nc.vector.tensor_copy(v_res[b][:, pos0:pos0 + NW], vps[:])
                qc = []
                for m in range(QH):
                    t = spool.tile([128, NW], BF16, tag=f"qc{m}", name=f"qc{m}")
                    if m % 2 == 0:
                        nc.scalar.copy(t[:], qps[m][:])
                    else:
                        nc.vector.tensor_copy(t[:], qps[m][:])
                    qc.append(t)
                if n < NG - 1:
                    for m in range(QH):
                        rope_emit(qc[m], q_res[b][m][:, pos0:pos0 + NW], pos0)
                    rope_emit(kc, k_res[b][:, pos0:pos0 + NW], pos0)
                else:
                    # group 7's rope is deferred into phase 2 (the staging
                    # tiles are resident): its results aren't needed until
                    # the batch-1 attention units ~200us later, and keeping
                    # its 15 DVE ops off the queue here lets the first
                    # attention units' DVE work start immediately
                    g7_staged.extend(
                        [(qc[m], q_res[1][m]) for m in range(QH)]
                        + [(kc, k_res[1])])

        # ------------------------------------------------------------------
        # Phase 2: attention + output projection
        # ------------------------------------------------------------------
        with tc.tile_pool(name="pacc", bufs=2) as papool, \
             tc.tile_pool(name="attn", bufs=2) as atpool, \
             tc.tile_pool(name="smax", bufs=2) as smpool, \
             tc.tile_pool(name="ystage", bufs=2) as ypool, \
             tc.tile_pool(name="sps", bufs=3, space="PSUM") as spsum, \
             tc.tile_pool(name="avps", bufs=2, space="PSUM") as avpsum, \
             tc.tile_pool(name="normps", bufs=1, space="PSUM") as normps, \
             tc.tile_pool(name="yps", bufs=2, space="PSUM") as ypsum:

            def wo_gen(att_prev, b_prev, q0_prev, fine_dma=False):
                """Output projection for a finished q block, as a generator
                that yields once per PE matmul so the caller can interleave
                them into the attention stream."""
                for tcx in range(QB // 128):
                    tg0 = b_prev * S + q0_prev + tcx * 128
                    for half in range(2):
                        ysb = ypool.tile([128, D // 2], BF16, tag="ysb",
                                         name="ysb")
                        for dgh in range(4):
                            dg = half * 4 + dgh
                            yp = ypsum.tile([128, NW], F32, tag="yp", name="yp")
                            for f in range(QH):
                                nc.tensor.matmul(
                                    yp[:],
                                    att_prev[f][:, tcx * 128:(tcx + 1) * 128],
                                    wo_sb[:, f * D + dg * NW:f * D + (dg + 1) * NW],
                                    start=(f == 0), stop=(f == QH - 1))
                                yield
                            # split on ACT/DVE: shorter queue items mean the
                            # exp stream is never blocked long, and the yp
                            # bank frees faster. Asymmetric 192/320 split
                            # because ACT also carries all the exps.
                            nc.scalar.copy(
                                ysb[:, dgh * NW:dgh * NW + 192],
                                yp[:, 0:192])
                            nc.vector.tensor_copy(
                                ysb[:, dgh * NW + 192:(dgh + 1) * NW],
                                yp[:, 192:NW])
                            if fine_dma:
                                # tail of the kernel: ship each 512-col piece
                                # as soon as it's staged
                                nc.sync.dma_start(
                                    y[tg0:tg0 + 128,
                                      dg * NW:(dg + 1) * NW],
                                    ysb[:, dgh * NW:(dgh + 1) * NW])
                        if not fine_dma:
                            nc.sync.dma_start(
                                y[tg0:tg0 + 128,
                                  half * (D // 2):(half + 1) * (D // 2)],
                                ysb[:])

            def drain(gen, k):
                if gen is None:
                    return
                for _ in range(k):
                    try:
                        next(gen)
                    except StopIteration:
                        return

            units = [(b, qb, h) for b in range(B)
                     for qb in range(S // QB) for h in range(QH)]

            def u_nkt(u):
                return (u[1] + 1) * (QB // 128)

            def chunk_geom(u, c):
                vv = c - (u_nkt(u) - 4)
                if vv >= 0:
                    return vv * 128, (4 - vv) * 128  # qoff, width
                return 0, QB

            flat = [(i, c) for i, u in enumerate(units)
                    for c in range(u_nkt(u))]
            score_tiles = {}
            pt_tiles = {}
            pacc_cur = {}

            def emit_score(i, c):
                ub, uqb, uh = units[i]
                qoff, w = chunk_geom(units[i], c)
                t = spsum.tile([128, QB], F32, tag="stp", name="stp")
                nc.tensor.matmul(
                    t[:, 0:w], k_res[ub][:, c * 128:(c + 1) * 128],
                    q_res[ub][uh][:, uqb * QB + qoff:uqb * QB + qoff + w],
                    start=True, stop=True)
                score_tiles[(i, c)] = t

            def emit_score_pos(p):
                if p < len(flat):
                    i, c = flat[p]
                    if (i, c) not in score_tiles:
                        emit_score(i, c)

            def exp_stage(p):
                """exp + causal mask + denominator accumulation for flat[p].
                Runs 2 chunks ahead of the AV stage so the 687ns ACT exp is
                never on the PE's critical path."""
                if p >= len(flat):
                    return
                i2, c2 = flat[p]
                u2 = units[i2]
                nkt2 = u_nkt(u2)
                qoff2, w2 = chunk_geom(u2, c2)
                if (i2, c2) not in score_tiles:
                    emit_score(i2, c2)
                stp = score_tiles.pop((i2, c2))
                pt = ptpool.tile([128, QB], BF16, tag="pt", name="pt")
                nc.scalar.activation(pt[:, 0:w2], stp[:, 0:w2], EXP,
                                     scale=SCALE)
                if c2 >= nkt2 - 4:
                    # causal triangle: zero the below-diagonal block of the
                    # first 128 q columns (0/1 multiply on the DVE keeps the
                    # PE out of it entirely)
                    nc.vector.tensor_mul(pt[:, 0:128], pt[:, 0:128], tri01[:])
                if c2 == 0:
                    # bf16 accumulation: 2x DVE throughput; the denominator
                    # only needs ~3 significant digits (rel-err budget 2e-2)
                    pacc_cur[i2] = papool.tile([128, QB], BF16, tag="pacc",
                                               name="pacc")
                    nc.vector.tensor_copy(pacc_cur[i2][:], pt[:])
                else:
                    nc.vector.tensor_add(
                        pacc_cur[i2][:, qoff2:qoff2 + w2],
                        pacc_cur[i2][:, qoff2:qoff2 + w2], pt[:, 0:w2])
                pt_tiles[(i2, c2)] = pt

            # gpsimd-broadcast norm path, used for heads 0..QH-2 where the
            # whole 4-engine chain hides inside the next unit's stream.
            def make_norm(avp, smp, att_t):
                def norm():
                    s1 = smpool.tile([1, QB], F32, tag="s1", name="s1")
                    nc.vector.tensor_copy(s1[:], smp[0:1, :])
                    s_bc = smpool.tile([128, QB], F32, tag="s_bc")
                    nc.gpsimd.partition_broadcast(s_bc[:], s1[:])
                    r_bc = smpool.tile([128, QB], F32, tag="r_bc")
                    nc.vector.reciprocal_approx_fast(r_bc[:], s_bc[:])
                    nc.vector.tensor_mul(att_t[:], avp[:], r_bc[:])
                return norm

            # Group 7's deferred rope: all 10 half-swap DMAs issue upfront
            # (sync queue is empty here, and each head has its own xsw
            # buffer so there are no WAR waits); the 15 DVE ops then drip
            # in one per two attention iterations so they never back up the
            # DVE queue. Results are needed ~200us later (batch 1).
            g7_ops = []
            g7_pos0 = ((NG - 1) * NW) % S
            for g7_src, g7_dst in g7_staged:
                g7_xsw = g7xp.tile([128, NW], BF16, tag="xsw")
                nc.sync.dma_start(g7_xsw[0:64, :], g7_src[64:128, :])
                nc.sync.dma_start(g7_xsw[64:128, :], g7_src[0:64, :])
                g7_ref = {}

                def op_a(xsw=g7_xsw, ref=g7_ref):
                    t1 = g7tmp.tile([128, NW], BF16, tag="t1")
                    nc.vector.tensor_mul(t1[:], xsw[:],
                                         sin_s[:, g7_pos0:g7_pos0 + NW])
                    ref["t1"] = t1

                def op_b(src=g7_src, ref=g7_ref):
                    t2 = g7tmp.tile([128, NW], BF16, tag="t2")
                    nc.vector.tensor_mul(t2[:], src[:],
                                         cos_s[:, g7_pos0:g7_pos0 + NW])
                    ref["t2"] = t2

                def op_c(dst=g7_dst, ref=g7_ref):
                    nc.vector.tensor_add(dst[:, g7_pos0:g7_pos0 + NW],
                                         ref["t2"][:], ref["t1"][:])

                g7_ops.extend([op_a, op_b, op_c])

            pending = None
            deferred = None
            att = None
            emit_score_pos(0)
            emit_score_pos(1)
            emit_score_pos(2)
            exp_stage(0)
            exp_stage(1)

            pos = 0
            avp = None
            smp = None
            r1 = None
            rbc_ps = None
            for i, u in enumerate(units):
                b, qb, h = u
                nkt = u_nkt(u)
                V_b = V_bs[b]
                last_head = (h == QH - 1)
                if h == 0:
                    att = [atpool.tile([128, QB], BF16, tag=f"att{hh}",
                                       name=f"att{hh}") for hh in range(QH)]
                wo_per_chunk = -(-(QB // 128 * 8) // nkt)   # ceil

                for c in range(nkt):
                    exp_stage(pos + 2)
                    if c == 0:
                        avp = avpsum.tile([128, QB], F32, tag="avp",
                                          name="avp")
                    pt_t = pt_tiles.pop((i, c))
                    qoff, w = chunk_geom(u, c)
                    nc.tensor.matmul(
                        avp[:, qoff:qoff + w],
                        V_b[:, c * 128:(c + 1) * 128], pt_t[:, 0:w],
                        start=(c == 0), stop=(c == nkt - 1))
                    emit_score_pos(pos + 3)
                    if i == 0 and c < 4:
                        # batch 1's last 4 V transposes, deferred into the
                        # first attention unit: this stretch is ACT-bound
                        # (no wo drains yet) so the PE has spare cycles,
                        # and it shortens the phase-1 tail
                        if c == 0:
                            vt_sp = avpsum.tile([128, QB], BF16, tag="avp",
                                                name="avp")
                        sl = slice(c * 128, (c + 1) * 128)
                        nc.tensor.transpose(
                            vt_sp[:, sl],
                            v_res[1][:, (12 + c) * 128:(13 + c) * 128],
                            ident[:])
                        if c % 2 == 0:
                            nc.scalar.copy(
                                V_bs[1][:, (12 + c) * 128:(13 + c) * 128],
                                vt_sp[:, sl])
                        else:
                            nc.vector.tensor_copy(
                                V_bs[1][:, (12 + c) * 128:(13 + c) * 128],
                                vt_sp[:, sl])
                    if c == 1 and deferred is not None:
                        deferred()
                        deferred = None
                    if c == (nkt - 3 if last_head else nkt - 1):
                        # pacc is complete already (exp stage ran 2 ahead):
                        # denominator row-sums via one 512-row matmul. For
                        # non-last heads this sits at unit end so it never
                        # stalls the PE queue on a backlogged DVE (the
                        # deferred norm only needs it one unit later).
                        smp = normps.tile([128, QB], F32, tag="np", name="np")
                        nc.tensor.matmul(
                            smp[0:1, :], ones_t[:], pacc_cur[i][:],
                            start=True, stop=True)
                    if c == nkt - 2 and last_head:
                        # fast norm path pieces 1+2: stage the row sums to
                        # SBUF (fp32r), then an fp32r PE outer-product
                        # broadcast of the *denominator*
                        r1 = smpool.tile([1, QB], F32R, tag="s1", name="s1")
                        nc.vector.tensor_copy(r1[:], smp[0:1, :])
                        rbc_ps = normps.tile([128, QB], F32, tag="np",
                                             name="np")
                        nc.tensor.matmul(
                            rbc_ps[:], ones_row[:], r1[:],
                            start=True, stop=True)
                    drain(pending, wo_per_chunk)
                    # start only after the swap DMAs are certainly resident
                    # (batch-1 units begin at pos 160; last op lands ~119)
                    if g7_ops and pos >= 32 and pos % 6 == 5:
                        g7_ops.pop(0)()
                    pos += 1
                if deferred is not None:
                    deferred()
                    deferred = None
                if not last_head:
                    deferred = make_norm(avp, smp, att[h])
                else:
                    # fast norm path piece 3: per-tcx reciprocal+mul so the
                    # first 128 columns of att[3] are ready ~1.8us after the
                    # last AV instead of ~2.9us via the gpsimd chain
                    rbc_sb = smpool.tile([128, QB], F32, tag="r_bc")
                    for t4 in range(4):
                        sl = slice(t4 * 128, (t4 + 1) * 128)
                        nc.vector.reciprocal_approx_fast(rbc_sb[:, sl],
                                                         rbc_ps[:, sl])
                        nc.vector.tensor_mul(att[h][:, sl], avp[:, sl],
                                             rbc_sb[:, sl])
                    drain(pending, 10 ** 9)
                    pending = wo_gen(att, b, qb * QB,
                                     fine_dma=(i == len(units) - 1))
            drain(pending, 10 ** 9)
    nc.compile()
    return nc


_program = None


def _get_program():
    global _program
    if _program is None:
        _program = _build_program()
    return _program


def kernel(**inputs) -> np.ndarray:
    x = np.asarray(inputs["x"], dtype=np.float32)
    wq = np.asarray(inputs["wq"], dtype=np.float32)
    wk = np.asarray(inputs["wk"], dtype=np.float32)
    wv = np.asarray(inputs["wv"], dtype=np.float32)
    wo = np.asarray(inputs["wo"], dtype=np.float32)
    cos = np.asarray(inputs["freqs_cos"], dtype=np.float32)
    sin = np.asarray(inputs["freqs_sin"], dtype=np.float32)
    start_pos = int(np.asarray(inputs.get("start_pos", 0)))
    assert start_pos == 0, "kernel specialized for start_pos == 0"

    # Even/odd RoPE pair split within each head's 128 features.
    perm = np.concatenate([np.arange(0, HD, 2), np.arange(1, HD, 2)])

    xT = np.ascontiguousarray(x.reshape(T, D).T.astype(NPBF16))
    cosT = cos.T                                   # [64, S]
    sinT = sin.T
    ropc = np.ascontiguousarray(
        np.concatenate([cosT, cosT], axis=0).astype(NPBF16))
    rops = np.ascontiguousarray(
        np.concatenate([-sinT, sinT], axis=0).astype(NPBF16))
    rr, cc = np.meshgrid(np.arange(128), np.arange(128), indexing="ij")
    tri01in = (cc >= rr).astype(np.float32).astype(NPBF16)

    in_maps = []
    for c in range(N_CORES):
        wq_c = wq[c * FL:(c + 1) * FL].reshape(QH, HD, D)[:, perm, :].reshape(FL, D)
        wk_c = wk[c * HD:(c + 1) * HD][perm, :]
        wv_c = wv[c * HD:(c + 1) * HD]
        wo_c = wo[:, c * FL:(c + 1) * FL]
        in_maps.append({
            "xT": xT,
            "idin": np.eye(128, dtype=np.float32).astype(NPBF16),
            "wqT": np.ascontiguousarray(wq_c.T.astype(NPBF16)),
            "wkT": np.ascontiguousarray(wk_c.T.astype(NPBF16)),
            "wvT": np.ascontiguousarray(wv_c.T.astype(NPBF16)),
            "woT": np.ascontiguousarray(wo_c.T.astype(NPBF16)),
            "ropc": ropc,
            "rops": rops,
            "onesin": np.ones((128, 1), dtype=np.float32).astype(NPBF16),
            "onesrow": np.ones((1, 128), dtype=np.float32),
            "tri01in": tri01in,
        })

    nc = _get_program()
    trace = bool(int(os.environ.get("GQA_TRACE", "0")))
    kwargs = {}
    if trace:
        tmpdir = os.environ.get("GQA_TRACE_DIR") or None
        kwargs = dict(trace=True, tmpdir=tmpdir, trace_cores=[0])
    res = run_bass_kernel_spmd(nc, in_maps, list(range(N_CORES)), **kwargs)
    kernel.last_results = res

    acc = np.zeros((T, D), dtype=np.float32)
    for c in range(N_CORES):
        acc += np.asarray(res.results[c]["y"]).astype(np.float32)
    return acc.reshape(B, S, D)
